# revision 28
# baseline (speedup 1.0000x reference)
"""Trainium2 Bass kernel for nn_CWGDN (dense_cnn): LN -> temporal pin conv ->
dynamic depthwise conv (w/ pooled kernel-generator branch) -> gate -> temporal
pout conv + residual.

Sharding: 16 (b,t) instances over 8 cores (2 each), ONE SPMD launch.
Each core computes gated(t0), gated(t0+1) and the pout partial products its
own gated slices contribute to; the t-halo terms are exported (zpn) and the
host sums partials + residual. No second launch, no halo recompute.

Engine split per core:
  PE : LN stats matmuls, pin matmuls, tok matmuls, the full-res 3x3 dynamic
       depthwise conv as 9 diagonal-weight matmuls w/ PSUM accumulation,
       pout partial matmuls.
  DVE: LN scalar math, x*rsqrt scale, avg/max pools, 5/9 taps of each 64x64
       dwconv layer, gating, diag-weight construction.
  GpSimd: remaining 3+1... 3 taps of each 64x64 layer (independent STT chain).
  Scalar: Square for stats, all PSUM drains (w/ folded biases).

The 32x32 conv stack + global mean collapses on the host into a per-channel
32x32 weight map (linear functional) -> one STT w/ accum on device; its bias
term folds into tok_b.

LayerNorm folds into the pin matmul: x is pre-scaled by r=rsqrt(var+eps)
(per-pixel, via a DMA-broadcast row) and the -mu*r rank-1 terms ride as 3
extra contraction rows; the lnb bias rides in the drain activations.
"""
import sys

sys.path.insert(0, "/opt/trn_rl_repo")

import numpy as np
import ml_dtypes

import concourse.bass as bass
import concourse.tile as tile
from concourse import bacc, mybir
from concourse.bass_utils import run_bass_kernel_spmd

BF = ml_dtypes.bfloat16
F32 = mybir.dt.float32
BF16 = mybir.dt.bfloat16
AL = mybir.AluOpType
ACTF = mybir.ActivationFunctionType

B, T, C, H, W = 2, 8, 64, 128, 128
HID = 128
S = H * W  # 16384
K = 3
EPS = 1e-5
TAPS_V = ()  # dyn-conv taps done on vector engine (rest on PE)

_cache = {}
TRACE = False
PROF = {}


def _build(dbg=False):
    scratch_kind = "ExternalOutput" if dbg else "Internal"
    nc = bacc.Bacc("TRN2", target_bir_lowering=False, debug=False, num_devices=8)
    xh = nc.dram_tensor("xh", [4, C, S], BF16, kind="ExternalInput")
    w1p = nc.dram_tensor("w1p", [2, 2, 128, 128], BF16, kind="ExternalInput")
    w1lo = nc.dram_tensor("w1lo", [2, 2, 67, 128], BF16, kind="ExternalInput")
    pbias = nc.dram_tensor("pbias", [128, 4], F32, kind="ExternalInput")
    bw = nc.dram_tensor("bw", [128, 27], F32, kind="ExternalInput")
    bb = nc.dram_tensor("bb", [128, 3], F32, kind="ExternalInput")
    umap = nc.dram_tensor("umap", [128, 1024], BF16, kind="ExternalInput")
    tokw = nc.dram_tensor("tokw", [9, 128, 128], BF16, kind="ExternalInput")
    tokb = nc.dram_tensor("tokb", [128, 9], F32, kind="ExternalInput")
    dwb = nc.dram_tensor("dwb", [128, 1], F32, kind="ExternalInput")
    w2 = nc.dram_tensor("w2", [3, 128, 64], BF16, kind="ExternalInput")
    ident = nc.dram_tensor("ident", [128, 128], BF16, kind="ExternalInput")
    zab = nc.dram_tensor("zab", [128, S], BF16, kind="ExternalOutput")
    zpn = nc.dram_tensor("zpn", [128, S], BF16, kind="ExternalOutput")
    # internal DRAM scratch
    scr_sq = nc.dram_tensor("scr_sq", [2, 2, 2 * S], BF16, kind=scratch_kind)
    r_scr = nc.dram_tensor("r_scr", [4, S], BF16, kind=scratch_kind)
    mur_scr = nc.dram_tensor("mur_scr", [4, S], BF16, kind=scratch_kind)
    x2d = nc.dram_tensor("x2d", [2, 128, S], BF16, kind=scratch_kind)
    g_dbg = nc.dram_tensor("g_dbg", [2, 128, S], BF16,
                           kind=scratch_kind) if dbg else None
    k_dbg = nc.dram_tensor("k_dbg", [2, 128, 16], F32,
                           kind=scratch_kind) if dbg else None
    dg_dbg = nc.dram_tensor("dg_dbg", [9, 128, 128], BF16,
                            kind=scratch_kind) if dbg else None
    xi_dbg = nc.dram_tensor("xi_dbg", [2, 128, S], BF16,
                            kind=scratch_kind) if dbg else None

    with tile.TileContext(nc, pool_alloc_mode="queue") as tc:
        with tc.tile_pool(name="wp", bufs=1) as wp:
            w1p_sb, w1lo_sb = [], []
            for j in range(2):
                w1p_sb.append([])
                w1lo_sb.append([])
                for oh in range(2):
                    tp = wp.tile([128, 128], BF16, tag=f"w1p{j}{oh}")
                    nc.sync.dma_start(tp[:], w1p[j, oh])
                    w1p_sb[j].append(tp)
                    tl = wp.tile([67, 128], BF16, tag=f"w1lo{j}{oh}")
                    nc.sync.dma_start(tl[:], w1lo[j, oh])
                    w1lo_sb[j].append(tl)
            pb_sb = wp.tile([128, 4], F32, tag="pb")
            nc.sync.dma_start(pb_sb[:], pbias[:])
            bw_sb = wp.tile([128, 27], F32, tag="bw")
            nc.sync.dma_start(bw_sb[:], bw[:])
            bb_sb = wp.tile([128, 3], F32, tag="bb")
            nc.sync.dma_start(bb_sb[:], bb[:])
            um_sb = wp.tile([128, 1024], BF16, tag="um")
            nc.sync.dma_start(um_sb[:], umap[:])
            tokw_sb = []
            for k in range(9):
                tk = wp.tile([128, 128], BF16, tag=f"tokw{k}")
                nc.sync.dma_start(tk[:], tokw[k])
                tokw_sb.append(tk)
            tokb_sb = wp.tile([128, 9], F32, tag="tokb")
            nc.sync.dma_start(tokb_sb[:], tokb[:])
            dwb_sb = wp.tile([128, 1], F32, tag="dwb")
            nc.sync.dma_start(dwb_sb[:], dwb[:])
            w2_sb = []
            for tau in range(3):
                tw2 = wp.tile([128, 64], BF16, tag=f"w2{tau}")
                nc.sync.dma_start(tw2[:], w2[tau])
                w2_sb.append(tw2)
            id_sb = wp.tile([128, 128], BF16, tag="id")
            nc.sync.dma_start(id_sb[:], ident[:])
            i2 = wp.tile([128, 2], BF16, tag="i2")
            nc.gpsimd.memset(i2[:, :], 0.0)
            nc.gpsimd.memset(i2[0:64, 0:1], 1.0)
            nc.gpsimd.memset(i2[64:128, 1:2], 1.0)
            eps_t = wp.tile([128, 1], F32, tag="eps")
            nc.gpsimd.memset(eps_t[:, :], EPS)

            with tc.tile_pool(name="cp0", bufs=1) as cp0:
                x1p = [cp0.tile([128, 130 * 130], BF16, tag=f"x1_{j}",
                                name=f"x1t{j}") for j in range(2)]
                xpv = [x1p[j][:].rearrange("p (h w) -> p h w", h=130)
                       for j in range(2)]
                for j in range(2):
                    nc.gpsimd.memset(xpv[j][:, 0:1, :], 0.0)
                    nc.gpsimd.memset(xpv[j][:, 129:130, :], 0.0)
                    nc.gpsimd.memset(xpv[j][:, 1:129, 0:1], 0.0)
                    nc.gpsimd.memset(xpv[j][:, 1:129, 129:130], 0.0)
                pab = [cp0.tile([128, 66 * 66], BF16, tag=f"pp{n}",
                                name=f"pp{n}") for n in range(2)]
                pabv = [t[:].rearrange("p (h w) -> p h w", h=66) for t in pab]
                for v in pabv:
                    nc.gpsimd.memset(v[:, 0:1, :], 0.0)
                    nc.gpsimd.memset(v[:, 65:66, :], 0.0)
                    nc.gpsimd.memset(v[:, 1:65, 0:1], 0.0)
                    nc.gpsimd.memset(v[:, 1:65, 65:66], 0.0)
                tmpB = cp0.tile([128, 4096], BF16, tag="tmpB")
                q32 = cp0.tile([128, 1024], BF16, tag="q32")
                pooled = cp0.tile([128, 2], F32, tag="pooled")
                pool16 = cp0.tile([128, 2], BF16, tag="pool16")
                kern = cp0.tile([128, 9], F32, tag="kern")

                with tc.tile_pool(name="fp", bufs=1) as fp:
                    pairs = [fp.tile([128, S], BF16, tag=f"pair{p}",
                                     name=f"pair{p}") for p in range(2)]
                    # ---- stats + LN + scale, per pair ----
                    # Per 512-pixel chunk one (2,1024) PSUM tile holds the
                    # channel-sums of x (cols 0:512) and x^2 (cols 512:1024):
                    # same tile_position for both matmuls. scr_sq keeps that
                    # interleaved [S(512)|Q(512)] layout per chunk.
                    with tc.tile_pool(name="sp_", bufs=1) as spool, \
                         tc.tile_pool(name="ps_s", bufs=2, space="PSUM") as psa:
                        for p in range(2):
                            for c8 in range(8):
                                cs = slice(c8 * 2048, (c8 + 1) * 2048)
                                nc.sync.dma_start(
                                    pairs[p][:, cs],
                                    xh[2 * p: 2 * p + 2, :, cs].rearrange(
                                        "s c f -> (s c) f"))
                        for ch in range(16):
                            n0 = ch * 1024
                            for p in range(2):
                                sq = spool.tile([128, 1024], BF16, tag="sq",
                                                bufs=2)
                                nc.vector.tensor_tensor(
                                    sq[:], pairs[p][:, n0: n0 + 1024],
                                    pairs[p][:, n0: n0 + 1024], op=AL.mult)
                                stg = spool.tile([2, 2048], BF16,
                                                 tag="stg", bufs=2)
                                for c4 in range(2):
                                    nd = n0 + c4 * 512
                                    ps = psa.tile([2, 1024], F32, tag="st")
                                    nc.tensor.matmul(
                                        ps[:, 0:512], i2[:],
                                        pairs[p][:, nd: nd + 512],
                                        start=True, stop=True)
                                    nc.tensor.matmul(
                                        ps[:, 512:1024], i2[:],
                                        sq[:, c4 * 512: (c4 + 1) * 512],
                                        start=True, stop=True)
                                    if c4 == 0:
                                        nc.scalar.copy(stg[:, 0:1024], ps[:])
                                    else:
                                        nc.vector.tensor_copy(
                                            stg[:, 1024:2048], ps[:])
                                nc.sync.dma_start(
                                    scr_sq[p][:, 2 * n0: 2 * n0 + 2048],
                                    stg[:])
                        for p in range(2):
                            # LN math in pixel-spread layout: sp[q, g, sq, f]
                            # with pixel = (c,q2,f) c=32 chunks, q=(c,q2)
                            sp = spool.tile([128, 512], BF16, tag=f"sp{p}")
                            spv4 = sp[:].rearrange("p (g t f) -> p g t f",
                                                   g=2, t=2)
                            scv = scr_sq[p].rearrange(
                                "g (c t q2 f) -> g c t q2 f",
                                c=32, t=2, q2=4)
                            for g in range(2):
                                for t in range(2):
                                    nc.sync.dma_start(
                                        spv4[:, g, t, :],
                                        scv[g, :, t, :, :])
                            mu = spool.tile([128, 256], F32, tag=f"mu{p}")
                            muv = mu[:].rearrange("p (g f) -> p g f", g=2)
                            nc.vector.tensor_scalar(
                                muv, spv4[:, :, 0, :], 1.0 / 64.0, None,
                                op0=AL.mult)
                            var = spool.tile([128, 256], F32, tag=f"var{p}")
                            varv = var[:].rearrange("p (g f) -> p g f", g=2)
                            nc.vector.tensor_scalar(
                                varv, spv4[:, :, 1, :], 1.0 / 64.0, None,
                                op0=AL.mult)
                            msq = spool.tile([128, 256], F32, tag=f"msq{p}")
                            nc.vector.tensor_tensor(msq[:], mu[:], mu[:],
                                                    op=AL.mult)
                            nc.vector.tensor_tensor(var[:], var[:], msq[:],
                                                    op=AL.subtract)
                            nc.scalar.activation(msq[:], var[:], ACTF.Sqrt,
                                                 bias=eps_t[:, :])  # std
                            nc.vector.reciprocal(var[:], msq[:])  # r
                            r16 = spool.tile([128, 256], BF16, tag=f"r16{p}")
                            nc.vector.tensor_copy(r16[:], var[:])
                            nc.vector.tensor_tensor(mu[:], mu[:], var[:],
                                                    op=AL.mult)  # mu*r
                            m16 = spool.tile([128, 256], BF16, tag=f"m16{p}")
                            nc.vector.tensor_copy(m16[:], mu[:])
                            nc.sync.dma_start(
                                r_scr[2 * p: 2 * p + 2].rearrange(
                                    "g (q f) -> q g f", q=128),
                                r16[:].rearrange("p (g f) -> p g f", g=2))
                            nc.sync.dma_start(
                                mur_scr[2 * p: 2 * p + 2].rearrange(
                                    "g (q f) -> q g f", q=128),
                                m16[:].rearrange("p (g f) -> p g f", g=2))
                        # broadcast r back; scale x in place
                        for c4 in range(4):
                            n0 = c4 * 4096
                            for p in range(2):
                                rb = spool.tile([128, 4096], BF16, tag="rb")
                                for g in range(2):
                                    nc.sync.dma_start(
                                        rb[g * 64: (g + 1) * 64, :],
                                        r_scr[2 * p + g: 2 * p + g + 1,
                                              n0: n0 + 4096].broadcast_to(
                                                  (64, 4096)))
                                nc.vector.tensor_tensor(
                                    pairs[p][:, n0: n0 + 4096],
                                    pairs[p][:, n0: n0 + 4096],
                                    rb[:], op=AL.mult)

                    # ---- pin matmuls ----
                    # j0-oh0 runs first (full c8 sweep) so x1p[0] completes
                    # early and the V branch phase starts while PE finishes
                    # the remaining pin passes.
                    with tc.tile_pool(name="pp", bufs=1) as pp, \
                         tc.tile_pool(name="ps_b", bufs=2, space="PSUM") as psb:

                        def lo_fetch(j):
                            lo = pp.tile([67, S], BF16, tag="lo")
                            if j == 0:
                                nc.sync.dma_start(lo[0:64, :],
                                                  pairs[1][0:64, :])
                            else:
                                nc.sync.dma_start(lo[0:64, :],
                                                  pairs[0][64:128, :])
                            nc.sync.dma_start(lo[64:67, :],
                                              mur_scr[j: j + 3, :])
                            return lo

                        def pin_pass(j, oh, c8, lo):
                            cs = slice(c8 * 2048, (c8 + 1) * 2048)
                            ps = psb.tile([128, 2048], F32, tag="piny")
                            for c4 in range(4):
                                s5 = slice(c4 * 512, (c4 + 1) * 512)
                                nc.tensor.matmul(
                                    ps[:, s5], w1p_sb[j][oh],
                                    pairs[j][:, c8 * 2048 + c4 * 512:
                                             c8 * 2048 + (c4 + 1) * 512],
                                    start=True, stop=False)
                            for c4 in range(4):
                                s5 = slice(c4 * 512, (c4 + 1) * 512)
                                nc.tensor.matmul(
                                    ps[:, s5], w1lo_sb[j][oh],
                                    lo[:, c8 * 2048 + c4 * 512:
                                       c8 * 2048 + (c4 + 1) * 512],
                                    start=False, stop=True)
                            if oh == 0:
                                r0 = 1 + c8 * 16
                                dst = xpv[j][:, r0: r0 + 16, 1: 1 + 128]
                                nc.scalar.activation(
                                    dst,
                                    ps[:].rearrange("p (h w) -> p h w", h=16),
                                    ACTF.Identity,
                                    bias=pb_sb[:, 2 * j: 2 * j + 1])
                            else:
                                stg2 = pp.tile([128, 2048], BF16, tag="stg2",
                                               bufs=2)
                                nc.scalar.activation(
                                    stg2[:], ps[:], ACTF.Identity,
                                    bias=pb_sb[:, 2 * j + 1: 2 * j + 2])
                                nc.sync.dma_start(x2d[j][:, cs], stg2[:])

                        lo0 = lo_fetch(0)
                        for c8 in range(8):
                            pin_pass(0, 0, c8, lo0)
                        for c8 in range(8):
                            pin_pass(0, 1, c8, lo0)
                        lo1 = lo_fetch(1)
                        for c8 in range(8):
                            pin_pass(1, 0, c8, lo1)
                        for c8 in range(8):
                            pin_pass(1, 1, c8, lo1)

                # ---- branch + dynconv + gate + pout, pair pool closed ----
                with tc.tile_pool(name="lp", bufs=1) as lp:
                    acc = [lp.tile([128, S], BF16, tag=f"acc{j}",
                                   name=f"acc{j}") for j in range(2)]
                    dg = [lp.tile([128, 128], BF16, tag=f"dg{k}",
                                  name=f"dg{k}") for k in range(9)]
                    psd_ctx = tc.tile_pool(name="ps_d", bufs=1, space="PSUM")
                    psd = psd_ctx.__enter__()

                    def branch(j):
                        x1v = xpv[j][:, 1:129, 1:129]
                        pav = pabv[0]
                        # avgpool2 (sum; 0.25 folded into layer-0 weights)
                        tb = tmpB[:].rearrange("p (h w) -> p h w", h=64)
                        for hf in range(2):
                            xe = x1v[:, 64 * hf: 64 * hf + 64, :].rearrange(
                                "p h (w2 two) -> p h w2 two", two=2)
                            nc.vector.tensor_tensor(
                                tb, xe[:, :, :, 0], xe[:, :, :, 1], op=AL.add)
                            te = tb.rearrange(
                                "p (h2 two) w -> p h2 two w", two=2)
                            nc.vector.tensor_tensor(
                                pav[:, 1 + 32 * hf: 1 + 32 * hf + 32, 1:65],
                                te[:, :, 0, :], te[:, :, 1, :], op=AL.add)
                        cur = 0
                        for l in range(3):
                            nxt = 1 - cur
                            ip = pabv[cur]
                            oi = pabv[nxt][:, 1:65, 1:65]
                            kap = bw_sb[:, l * 9: l * 9 + 9]
                            nc.vector.tensor_scalar(
                                oi, ip[:, 1:65, 1:65], kap[:, 4: 5],
                                bb_sb[:, l: l + 1], op0=AL.mult, op1=AL.add)
                            for t in (0, 1, 2, 3, 5, 6, 7, 8):
                                ky, kx = t // 3, t % 3
                                src = ip[:, ky: ky + 64, kx: kx + 64]
                                nc.vector.tensor_scalar(
                                    tb, src, kap[:, t: t + 1], None,
                                    op0=AL.mult)
                                nc.vector.tensor_tensor(oi, oi, tb, op=AL.add)
                            cur = nxt
                        # maxpool2 -> q32
                        ci = pabv[cur][:, 1:65, 1:65]
                        ce = ci.rearrange("p h (w2 two) -> p h w2 two", two=2)
                        tm = tmpB[:, 0:2048].rearrange("p (h w) -> p h w",
                                                       h=64)
                        nc.vector.tensor_tensor(tm, ce[:, :, :, 0],
                                                ce[:, :, :, 1], op=AL.max)
                        tme = tm.rearrange("p (h2 two) w -> p h2 two w", two=2)
                        q3v = q32[:].rearrange("p (h w) -> p h w", h=32)
                        nc.vector.tensor_tensor(q3v, tme[:, :, 0, :],
                                                tme[:, :, 1, :], op=AL.max)
                        # collapsed 32x32 stack: pooled = <umap, q32>
                        nc.vector.scalar_tensor_tensor(
                            tmpB[:, 2048: 3072], q32[:], 1.0, um_sb[:],
                            op0=AL.mult, op1=AL.mult,
                            accum_out=pooled[:, j: j + 1])
                        nc.vector.tensor_copy(pool16[:, j: j + 1],
                                              pooled[:, j: j + 1])

                    def tok_and_diag(j):
                        psk = psd.tile([128, 512], F32, tag="psk")
                        for k in range(9):
                            nc.tensor.matmul(psk[:, k: k + 1], tokw_sb[k],
                                             pool16[:, j: j + 1],
                                             start=True, stop=True)
                        nc.scalar.copy(kern[:], psk[:, 0:9])
                        nc.vector.tensor_tensor(kern[:], kern[:], tokb_sb[:],
                                                op=AL.add)
                        if dbg:
                            nc.sync.dma_start(k_dbg[j][:, 0:9], kern[:])
                            nc.sync.dma_start(k_dbg[j][:, 9:10],
                                              pooled[:, j: j + 1])
                        for k in range(9):
                            nc.vector.tensor_scalar(
                                dg[k][:], id_sb[:], kern[:, k: k + 1], None,
                                op0=AL.mult)
                        if dbg and j == 0:
                            for k in range(9):
                                nc.sync.dma_start(dg_dbg[k], dg[k][:])
                            nc.sync.dma_start(
                                xi_dbg[j].rearrange("c (h w) -> c h w",
                                                    h=128),
                                xpv[j][:, 1:129, 1:129])

                    def dynconv(j):
                        pe_taps = [t for t in range(9) if t not in TAPS_V]
                        for dc in range(16):
                            r0 = 8 * dc
                            cs = slice(dc * 1024, (dc + 1) * 1024)
                            pd = psd.tile([128, 1024], F32, tag="pd", bufs=2)
                            for ti, t in enumerate(pe_taps):
                                ky, kx = t // 3, t % 3
                                for m in range(2):
                                    rhs = xpv[j][:, ky + r0 + 4 * m:
                                                 ky + r0 + 4 * m + 4,
                                                 kx: kx + 128]
                                    nc.tensor.matmul(
                                        pd[:, m * 512: (m + 1) * 512],
                                        dg[t], rhs,
                                        start=(ti == 0),
                                        stop=(ti == len(pe_taps) - 1))
                            nc.scalar.activation(acc[j][:, cs], pd[:],
                                                 ACTF.Identity,
                                                 bias=dwb_sb[:, :])

                    def vtaps_gate(j):
                        for dc in range(16):
                            r0 = 8 * dc
                            cs = slice(dc * 1024, (dc + 1) * 1024)
                            for t in TAPS_V:
                                ky, kx = t // 3, t % 3
                                vt = lp.tile([128, 1024], BF16, tag="vt",
                                             bufs=2)
                                src = xpv[j][:, ky + r0: ky + r0 + 8,
                                             kx: kx + 128]
                                nc.vector.tensor_scalar(
                                    vt[:].rearrange("p (h w) -> p h w", h=8),
                                    src, kern[:, t: t + 1], None, op0=AL.mult)
                                nc.vector.tensor_tensor(
                                    acc[j][:, cs], acc[j][:, cs], vt[:],
                                    op=AL.add)
                            x2t = lp.tile([128, 1024], BF16, tag="x2t",
                                          bufs=3)
                            nc.sync.dma_start(x2t[:], x2d[j][:, cs])
                            nc.gpsimd.tensor_tensor(
                                acc[j][:, cs], acc[j][:, cs], x2t[:],
                                op=AL.mult)

                    branch(0)
                    tok_and_diag(0)
                    branch(1)  # V work queued behind branch(0); overlaps PE dyn(0)
                    dynconv(0)
                    vtaps_gate(0)
                    tok_and_diag(1)
                    dynconv(1)
                    vtaps_gate(1)
                    if dbg:
                        for j in range(2):
                            nc.sync.dma_start(g_dbg[j], acc[j][:])
                    psd_ctx.__exit__(None, None, None)

                    # pout partials: zab rows 0:64 = W2[1]g0+W2[2]g1 (out t0),
                    # rows 64:128 = W2[0]g0+W2[1]g1 (out t0+1);
                    # zpn rows 0:64 = W2[2]g0 (export t0-1), 64:128 = W2[0]g1.
                    with tc.tile_pool(name="ps_z", bufs=2, space="PSUM") as psz:
                        for zc in range(16):
                            cs = slice(zc * 1024, (zc + 1) * 1024)
                            zt = psz.tile([128, 1024], F32, tag="zt")
                            pt = psz.tile([128, 1024], F32, tag="pt")
                            for c4 in range(2):
                                s5 = slice(c4 * 512, (c4 + 1) * 512)
                                m5 = slice(zc * 1024 + c4 * 512,
                                           zc * 1024 + (c4 + 1) * 512)
                                nc.tensor.matmul(zt[64:128, s5], w2_sb[0],
                                                 acc[0][:, m5],
                                                 start=True, stop=False)
                                nc.tensor.matmul(pt[64:128, s5], w2_sb[0],
                                                 acc[1][:, m5],
                                                 start=True, stop=True)
                            for c4 in range(2):
                                s5 = slice(c4 * 512, (c4 + 1) * 512)
                                m5 = slice(zc * 1024 + c4 * 512,
                                           zc * 1024 + (c4 + 1) * 512)
                                nc.tensor.matmul(zt[64:128, s5], w2_sb[1],
                                                 acc[1][:, m5],
                                                 start=False, stop=True)
                                nc.tensor.matmul(zt[0:64, s5], w2_sb[1],
                                                 acc[0][:, m5],
                                                 start=True, stop=False)
                            for c4 in range(2):
                                s5 = slice(c4 * 512, (c4 + 1) * 512)
                                m5 = slice(zc * 1024 + c4 * 512,
                                           zc * 1024 + (c4 + 1) * 512)
                                nc.tensor.matmul(zt[0:64, s5], w2_sb[2],
                                                 acc[1][:, m5],
                                                 start=False, stop=True)
                                nc.tensor.matmul(pt[0:64, s5], w2_sb[2],
                                                 acc[0][:, m5],
                                                 start=True, stop=True)
                            za = lp.tile([128, 1024], BF16, tag="za", bufs=2)
                            nc.scalar.copy(za[:], zt[:])
                            nc.sync.dma_start(zab[:, cs], za[:])
                            zp = lp.tile([128, 1024], BF16, tag="zp", bufs=2)
                            nc.vector.tensor_copy(zp[:], pt[:])
                            nc.sync.dma_start(zpn[:, cs], zp[:])
    nc.compile()
    return nc


def _xcorr_same(x, k):
    h, w = x.shape
    xp = np.zeros((h + 2, w + 2), x.dtype)
    xp[1: h + 1, 1: w + 1] = x
    y = np.zeros_like(x)
    for a in range(3):
        for b in range(3):
            y += k[a, b] * xp[a: a + h, b: b + w]
    return y


def _prep_weights(ln_w, ln_b, pin_w, pout_w, b1_w, b1_b, b2_w, b2_b, tok_w,
                  tok_b, dw_bias):
    pw = np.asarray(pin_w)[:, :, :, 0, 0].astype(np.float64)  # (256, 64, 3)
    lnw = np.asarray(ln_w).astype(np.float64)
    lnb = np.asarray(ln_b).astype(np.float64)
    W1 = [(pw[:, :, t] * lnw[None, :]).T for t in range(3)]  # (64, 256) each
    s1 = [(pw[:, :, t] * lnw[None, :]).sum(1) for t in range(3)]  # (256,)
    bias1 = [pw[:, :, t] @ lnb for t in range(3)]  # (256,)
    w1p = np.zeros((2, 2, 128, 128), np.float32)
    w1lo = np.zeros((2, 2, 67, 128), np.float32)
    for j in range(2):
        tA, tB = (0, 1) if j == 0 else (1, 2)
        tlo = 2 if j == 0 else 0
        # lo rows 64:67 hold mur slices (j..j+3) = taus (0,1,2) in order
        for oh in range(2):
            ohs = slice(oh * 128, (oh + 1) * 128)
            w1p[j, oh, 0:64] = W1[tA][:, ohs]
            w1p[j, oh, 64:128] = W1[tB][:, ohs]
            w1lo[j, oh, 0:64] = W1[tlo][:, ohs]
            for t in range(3):
                w1lo[j, oh, 64 + t] = -s1[t][ohs]
    bw = np.zeros((128, 27), np.float32)
    bb = np.zeros((128, 3), np.float32)
    b1w = np.asarray(b1_w)[:, :, 0]  # (3, 128, 3, 3)
    for l in range(3):
        bw[:, l * 9: l * 9 + 9] = b1w[l].reshape(128, 9)
        bb[:, l] = np.asarray(b1_b)[l]
    bw[:, 0:9] *= 0.25  # avgpool mean folded into layer-0 taps
    # collapse the 32x32 stack + global mean into <umap, q> + beta
    b2w = np.asarray(b2_w)[:, :, 0].astype(np.float64)  # (3, 128, 3, 3)
    b2b = np.asarray(b2_b).astype(np.float64)  # (3, 128)
    umap = np.zeros((128, 1024), np.float32)
    beta = np.zeros((128,), np.float64)
    for c in range(128):
        g = np.ones((32, 32), np.float64) / 1024.0
        for l in (2, 1, 0):
            g = _xcorr_same(g, np.flip(b2w[l, c]))
        umap[c] = g.reshape(-1)
        z = np.zeros((32, 32), np.float64)
        for l in range(3):
            z = _xcorr_same(z, b2w[l, c]) + b2b[l, c]
        beta[c] = z.mean()
    tokw = np.zeros((9, 128, 128), np.float32)
    tw = np.asarray(tok_w).astype(np.float64)  # (1152, 128)
    for k in range(9):
        tokw[k] = tw[k::9, :].T  # [h, c] = tok_w[c*9+k, h]
    tokb2 = (np.asarray(tok_b).astype(np.float64) + tw @ beta)
    tokb2 = tokb2.reshape(128, 9).astype(np.float32)
    w2 = np.zeros((3, 128, 64), np.float32)
    pow_ = np.asarray(pout_w)[:, :, :, 0, 0]  # (64, 128, 3)
    for t in range(3):
        w2[t] = pow_[:, :, t].T
    dwb = np.asarray(dw_bias).reshape(128, 1).astype(np.float32)
    ident = np.eye(128, dtype=np.float32)
    return (w1p.astype(BF), w1lo.astype(BF), bw, bb, umap.astype(BF),
            tokw.astype(BF), tokb2, w2.astype(BF), dwb, ident.astype(BF),
            np.array([np.asarray(b) for b in bias1]))


def kernel(x, ln_w, ln_b, pin_w, pout_w, b1_w, b1_b, b2_w, b2_b, tok_w, tok_b,
           dw_bias):
    x = np.asarray(x)
    (w1p, w1lo, bw, bb, umap, tokw, tokb2, w2, dwb, ident,
     bias1) = _prep_weights(ln_w, ln_b, pin_w, pout_w, b1_w, b1_b, b2_w, b2_b,
                            tok_w, tok_b, dw_bias)
    if "l1" not in _cache:
        _cache["l1"] = _build()

    xbf = x.astype(BF)  # (B, T, C, H, W)
    in_maps = []
    for i in range(8):
        b, t0 = i // 4, 2 * (i % 4)
        xh = np.zeros((4, C, S), BF)
        for k in range(4):
            t = t0 - 1 + k
            if 0 <= t < T:
                xh[k] = xbf[b, t].reshape(C, S)
        pbias = np.zeros((128, 4), np.float32)
        for j in range(2):
            for oh in range(2):
                s = 0.0
                for tau in range(3):
                    if 0 <= t0 + j - 1 + tau < T:
                        s = s + bias1[tau][oh * 128: (oh + 1) * 128]
                pbias[:, 2 * j + oh] = s
        in_maps.append({
            "xh": xh, "w1p": w1p, "w1lo": w1lo, "pbias": pbias, "bw": bw,
            "bb": bb, "umap": umap, "tokw": tokw, "tokb": tokb2, "dwb": dwb,
            "w2": w2, "ident": ident})
    r1 = run_bass_kernel_spmd(_cache["l1"], in_maps, core_ids=list(range(8)),
                              trace=TRACE)
    PROF["l1"] = r1

    out = x.astype(np.float32).copy()
    for i in range(8):
        b, t0 = i // 4, 2 * (i % 4)
        za = r1.results[i]["zab"].astype(np.float32).reshape(2, C, H, W)
        zp = r1.results[i]["zpn"].astype(np.float32).reshape(2, C, H, W)
        out[b, t0] += za[0]
        out[b, t0 + 1] += za[1]
        if t0 - 1 >= 0:
            out[b, t0 - 1] += zp[0]
        if t0 + 2 < T:
            out[b, t0 + 2] += zp[1]
    return out


# revision 29
# speedup vs baseline: 1.0398x; 1.0398x over previous
"""Trainium2 Bass kernel for nn_CWGDN (dense_cnn): LN -> temporal pin conv ->
dynamic depthwise conv (w/ pooled kernel-generator branch) -> gate -> temporal
pout conv + residual.

Sharding: 16 (b,t) instances over 8 cores (2 each), ONE SPMD launch.
Each core computes gated(t0), gated(t0+1) and the pout partial products its
own gated slices contribute to; the t-halo terms are exported (zpn) and the
host sums partials + residual. No second launch, no halo recompute.

Engine split per core:
  PE : LN stats matmuls, pin matmuls, tok matmuls, the full-res 3x3 dynamic
       depthwise conv as 9 diagonal-weight matmuls w/ PSUM accumulation,
       pout partial matmuls.
  DVE: LN scalar math, x*rsqrt scale, avg/max pools, 5/9 taps of each 64x64
       dwconv layer, gating, diag-weight construction.
  GpSimd: remaining 3+1... 3 taps of each 64x64 layer (independent STT chain).
  Scalar: Square for stats, all PSUM drains (w/ folded biases).

The 32x32 conv stack + global mean collapses on the host into a per-channel
32x32 weight map (linear functional) -> one STT w/ accum on device; its bias
term folds into tok_b.

LayerNorm folds into the pin matmul: x is pre-scaled by r=rsqrt(var+eps)
(per-pixel, via a DMA-broadcast row) and the -mu*r rank-1 terms ride as 3
extra contraction rows; the lnb bias rides in the drain activations.
"""
import sys

sys.path.insert(0, "/opt/trn_rl_repo")

import numpy as np
import ml_dtypes

import concourse.bass as bass
import concourse.tile as tile
from concourse import bacc, mybir
from concourse.bass_utils import run_bass_kernel_spmd

BF = ml_dtypes.bfloat16
F32 = mybir.dt.float32
BF16 = mybir.dt.bfloat16
AL = mybir.AluOpType
ACTF = mybir.ActivationFunctionType

B, T, C, H, W = 2, 8, 64, 128, 128
HID = 128
S = H * W  # 16384
K = 3
EPS = 1e-5
TAPS_V = ()  # dyn-conv taps done on vector engine (rest on PE)

_cache = {}
TRACE = False
PROF = {}


def _build(dbg=False):
    scratch_kind = "ExternalOutput" if dbg else "Internal"
    nc = bacc.Bacc("TRN2", target_bir_lowering=False, debug=False, num_devices=8)
    xh = nc.dram_tensor("xh", [4, C, S], BF16, kind="ExternalInput")
    w1p = nc.dram_tensor("w1p", [2, 2, 128, 128], BF16, kind="ExternalInput")
    w1lo = nc.dram_tensor("w1lo", [2, 2, 67, 128], BF16, kind="ExternalInput")
    pbias = nc.dram_tensor("pbias", [128, 4], F32, kind="ExternalInput")
    bw = nc.dram_tensor("bw", [128, 27], F32, kind="ExternalInput")
    bb = nc.dram_tensor("bb", [128, 3], F32, kind="ExternalInput")
    umap = nc.dram_tensor("umap", [128, 1024], BF16, kind="ExternalInput")
    tokw = nc.dram_tensor("tokw", [9, 128, 128], BF16, kind="ExternalInput")
    tokb = nc.dram_tensor("tokb", [128, 9], F32, kind="ExternalInput")
    dwb = nc.dram_tensor("dwb", [128, 1], F32, kind="ExternalInput")
    w2 = nc.dram_tensor("w2", [3, 128, 64], BF16, kind="ExternalInput")
    ident = nc.dram_tensor("ident", [128, 128], BF16, kind="ExternalInput")
    zab = nc.dram_tensor("zab", [128, S], BF16, kind="ExternalOutput")
    zpn = nc.dram_tensor("zpn", [128, S], BF16, kind="ExternalOutput")
    # internal DRAM scratch
    scr_sq = nc.dram_tensor("scr_sq", [2, 2, 2 * S], BF16, kind=scratch_kind)
    r_scr = nc.dram_tensor("r_scr", [4, S], BF16, kind=scratch_kind)
    mur_scr = nc.dram_tensor("mur_scr", [4, S], BF16, kind=scratch_kind)
    x2d = nc.dram_tensor("x2d", [2, 128, S], BF16, kind=scratch_kind)
    g_dbg = nc.dram_tensor("g_dbg", [2, 128, S], BF16,
                           kind=scratch_kind) if dbg else None
    k_dbg = nc.dram_tensor("k_dbg", [2, 128, 16], F32,
                           kind=scratch_kind) if dbg else None
    dg_dbg = nc.dram_tensor("dg_dbg", [9, 128, 128], BF16,
                            kind=scratch_kind) if dbg else None
    xi_dbg = nc.dram_tensor("xi_dbg", [2, 128, S], BF16,
                            kind=scratch_kind) if dbg else None

    with tile.TileContext(nc, pool_alloc_mode="queue") as tc:
        with tc.tile_pool(name="wp", bufs=1) as wp:
            w1p_sb, w1lo_sb = [], []
            for j in range(2):
                w1p_sb.append([])
                w1lo_sb.append([])
                for oh in range(2):
                    tp = wp.tile([128, 128], BF16, tag=f"w1p{j}{oh}")
                    nc.sync.dma_start(tp[:], w1p[j, oh])
                    w1p_sb[j].append(tp)
                    tl = wp.tile([67, 128], BF16, tag=f"w1lo{j}{oh}")
                    nc.sync.dma_start(tl[:], w1lo[j, oh])
                    w1lo_sb[j].append(tl)
            pb_sb = wp.tile([128, 4], F32, tag="pb")
            nc.sync.dma_start(pb_sb[:], pbias[:])
            bw_sb = wp.tile([128, 27], F32, tag="bw")
            nc.sync.dma_start(bw_sb[:], bw[:])
            bb_sb = wp.tile([128, 3], F32, tag="bb")
            nc.sync.dma_start(bb_sb[:], bb[:])
            um_sb = wp.tile([128, 1024], BF16, tag="um")
            nc.sync.dma_start(um_sb[:], umap[:])
            tokw_sb = []
            for k in range(9):
                tk = wp.tile([128, 128], BF16, tag=f"tokw{k}")
                nc.sync.dma_start(tk[:], tokw[k])
                tokw_sb.append(tk)
            tokb_sb = wp.tile([128, 9], F32, tag="tokb")
            nc.sync.dma_start(tokb_sb[:], tokb[:])
            dwb_sb = wp.tile([128, 1], F32, tag="dwb")
            nc.sync.dma_start(dwb_sb[:], dwb[:])
            w2_sb = []
            for tau in range(3):
                tw2 = wp.tile([128, 64], BF16, tag=f"w2{tau}")
                nc.sync.dma_start(tw2[:], w2[tau])
                w2_sb.append(tw2)
            id_sb = wp.tile([128, 128], BF16, tag="id")
            nc.sync.dma_start(id_sb[:], ident[:])
            i2 = wp.tile([128, 2], BF16, tag="i2")
            nc.gpsimd.memset(i2[:, :], 0.0)
            nc.gpsimd.memset(i2[0:64, 0:1], 1.0)
            nc.gpsimd.memset(i2[64:128, 1:2], 1.0)
            eps_t = wp.tile([128, 1], F32, tag="eps")
            nc.gpsimd.memset(eps_t[:, :], EPS)

            with tc.tile_pool(name="cp0", bufs=1) as cp0:
                x1p = [cp0.tile([128, 130 * 130], BF16, tag=f"x1_{j}",
                                name=f"x1t{j}") for j in range(2)]
                xpv = [x1p[j][:].rearrange("p (h w) -> p h w", h=130)
                       for j in range(2)]
                for j in range(2):
                    nc.gpsimd.memset(xpv[j][:, 0:1, :], 0.0)
                    nc.gpsimd.memset(xpv[j][:, 129:130, :], 0.0)
                    nc.gpsimd.memset(xpv[j][:, 1:129, 0:1], 0.0)
                    nc.gpsimd.memset(xpv[j][:, 1:129, 129:130], 0.0)
                pab = [cp0.tile([128, 66 * 66], BF16, tag=f"pp{n}",
                                name=f"pp{n}") for n in range(2)]
                pabv = [t[:].rearrange("p (h w) -> p h w", h=66) for t in pab]
                for v in pabv:
                    nc.gpsimd.memset(v[:, 0:1, :], 0.0)
                    nc.gpsimd.memset(v[:, 65:66, :], 0.0)
                    nc.gpsimd.memset(v[:, 1:65, 0:1], 0.0)
                    nc.gpsimd.memset(v[:, 1:65, 65:66], 0.0)
                tmpB = cp0.tile([128, 4096], BF16, tag="tmpB")
                q32 = cp0.tile([128, 1024], BF16, tag="q32")
                pooled = cp0.tile([128, 2], F32, tag="pooled")
                pool16 = cp0.tile([128, 2], BF16, tag="pool16")
                kern = cp0.tile([128, 9], F32, tag="kern")

                with tc.tile_pool(name="fp", bufs=1) as fp:
                    pairs = [fp.tile([128, S], BF16, tag=f"pair{p}",
                                     name=f"pair{p}") for p in range(2)]
                    # ---- stats + LN + scale, per pair ----
                    # Per 512-pixel chunk one (2,1024) PSUM tile holds the
                    # channel-sums of x (cols 0:512) and x^2 (cols 512:1024):
                    # same tile_position for both matmuls. scr_sq keeps that
                    # interleaved [S(512)|Q(512)] layout per chunk.
                    with tc.tile_pool(name="sp_", bufs=1) as spool, \
                         tc.tile_pool(name="ps_s", bufs=2, space="PSUM") as psa:
                        for p in range(2):
                            for c8 in range(8):
                                cs = slice(c8 * 2048, (c8 + 1) * 2048)
                                nc.sync.dma_start(
                                    pairs[p][:, cs],
                                    xh[2 * p: 2 * p + 2, :, cs].rearrange(
                                        "s c f -> (s c) f"))
                        for ch in range(16):
                            n0 = ch * 1024
                            for p in range(2):
                                sq = spool.tile([128, 1024], BF16, tag="sq",
                                                bufs=2)
                                nc.vector.tensor_tensor(
                                    sq[:], pairs[p][:, n0: n0 + 1024],
                                    pairs[p][:, n0: n0 + 1024], op=AL.mult)
                                stg = spool.tile([2, 2048], BF16,
                                                 tag="stg", bufs=2)
                                ps = psa.tile([2, 2048], F32, tag="st")
                                for c4 in range(2):
                                    nd = n0 + c4 * 512
                                    nc.tensor.matmul(
                                        ps[:, c4 * 1024: c4 * 1024 + 512],
                                        i2[:], pairs[p][:, nd: nd + 512],
                                        start=True, stop=True)
                                    nc.tensor.matmul(
                                        ps[:, c4 * 1024 + 512:
                                           c4 * 1024 + 1024],
                                        i2[:],
                                        sq[:, c4 * 512: (c4 + 1) * 512],
                                        start=True, stop=True)
                                if (ch + p) % 2 == 0:
                                    nc.scalar.copy(stg[:], ps[:])
                                else:
                                    nc.vector.tensor_copy(stg[:], ps[:])
                                nc.sync.dma_start(
                                    scr_sq[p][:, 2 * n0: 2 * n0 + 2048],
                                    stg[:])
                        for p in range(2):
                            # LN math in pixel-spread layout: sp[q, g, sq, f]
                            # with pixel = (c,q2,f) c=32 chunks, q=(c,q2)
                            sp = spool.tile([128, 512], BF16, tag=f"sp{p}")
                            spv4 = sp[:].rearrange("p (g t f) -> p g t f",
                                                   g=2, t=2)
                            scv = scr_sq[p].rearrange(
                                "g (c t q2 f) -> g c t q2 f",
                                c=32, t=2, q2=4)
                            for g in range(2):
                                for t in range(2):
                                    nc.sync.dma_start(
                                        spv4[:, g, t, :],
                                        scv[g, :, t, :, :])
                            mu = spool.tile([128, 256], F32, tag=f"mu{p}")
                            muv = mu[:].rearrange("p (g f) -> p g f", g=2)
                            nc.vector.tensor_scalar(
                                muv, spv4[:, :, 0, :], 1.0 / 64.0, None,
                                op0=AL.mult)
                            var = spool.tile([128, 256], F32, tag=f"var{p}")
                            varv = var[:].rearrange("p (g f) -> p g f", g=2)
                            nc.vector.tensor_scalar(
                                varv, spv4[:, :, 1, :], 1.0 / 64.0, None,
                                op0=AL.mult)
                            msq = spool.tile([128, 256], F32, tag=f"msq{p}")
                            nc.vector.tensor_tensor(msq[:], mu[:], mu[:],
                                                    op=AL.mult)
                            nc.vector.tensor_tensor(var[:], var[:], msq[:],
                                                    op=AL.subtract)
                            nc.scalar.activation(msq[:], var[:], ACTF.Sqrt,
                                                 bias=eps_t[:, :])  # std
                            nc.vector.reciprocal(var[:], msq[:])  # r
                            r16 = spool.tile([128, 256], BF16, tag=f"r16{p}")
                            nc.vector.tensor_copy(r16[:], var[:])
                            nc.vector.tensor_tensor(mu[:], mu[:], var[:],
                                                    op=AL.mult)  # mu*r
                            m16 = spool.tile([128, 256], BF16, tag=f"m16{p}")
                            nc.vector.tensor_copy(m16[:], mu[:])
                            nc.sync.dma_start(
                                r_scr[2 * p: 2 * p + 2].rearrange(
                                    "g (q f) -> q g f", q=128),
                                r16[:].rearrange("p (g f) -> p g f", g=2))
                            nc.sync.dma_start(
                                mur_scr[2 * p: 2 * p + 2].rearrange(
                                    "g (q f) -> q g f", q=128),
                                m16[:].rearrange("p (g f) -> p g f", g=2))
                        # broadcast r back; scale x in place
                        for c8 in range(8):
                            n0 = c8 * 2048
                            for p in range(2):
                                rb = spool.tile([128, 2048], BF16, tag="rb",
                                                bufs=2)
                                for g in range(2):
                                    nc.sync.dma_start(
                                        rb[g * 64: (g + 1) * 64, :],
                                        r_scr[2 * p + g: 2 * p + g + 1,
                                              n0: n0 + 2048].broadcast_to(
                                                  (64, 2048)))
                                nc.vector.tensor_tensor(
                                    pairs[p][:, n0: n0 + 2048],
                                    pairs[p][:, n0: n0 + 2048],
                                    rb[:], op=AL.mult)

                    # ---- pin matmuls ----
                    # j0-oh0 runs first (full c8 sweep) so x1p[0] completes
                    # early and the V branch phase starts while PE finishes
                    # the remaining pin passes.
                    with tc.tile_pool(name="pp", bufs=1) as pp, \
                         tc.tile_pool(name="ps_b", bufs=2, space="PSUM") as psb:

                        def lo_fetch(j):
                            lo = pp.tile([67, S], BF16, tag="lo")
                            for c8 in range(8):
                                cs = slice(c8 * 2048, (c8 + 1) * 2048)
                                if j == 0:
                                    nc.sync.dma_start(lo[0:64, cs],
                                                      pairs[1][0:64, cs])
                                else:
                                    nc.sync.dma_start(lo[0:64, cs],
                                                      pairs[0][64:128, cs])
                                nc.sync.dma_start(lo[64:67, cs],
                                                  mur_scr[j: j + 3, cs])
                            return lo

                        def pin_pass(j, oh, c8, lo):
                            cs = slice(c8 * 2048, (c8 + 1) * 2048)
                            ps = psb.tile([128, 2048], F32, tag="piny")
                            for c4 in range(4):
                                s5 = slice(c4 * 512, (c4 + 1) * 512)
                                nc.tensor.matmul(
                                    ps[:, s5], w1p_sb[j][oh],
                                    pairs[j][:, c8 * 2048 + c4 * 512:
                                             c8 * 2048 + (c4 + 1) * 512],
                                    start=True, stop=False)
                            for c4 in range(4):
                                s5 = slice(c4 * 512, (c4 + 1) * 512)
                                nc.tensor.matmul(
                                    ps[:, s5], w1lo_sb[j][oh],
                                    lo[:, c8 * 2048 + c4 * 512:
                                       c8 * 2048 + (c4 + 1) * 512],
                                    start=False, stop=True)
                            if oh == 0:
                                r0 = 1 + c8 * 16
                                dst = xpv[j][:, r0: r0 + 16, 1: 1 + 128]
                                nc.scalar.activation(
                                    dst,
                                    ps[:].rearrange("p (h w) -> p h w", h=16),
                                    ACTF.Identity,
                                    bias=pb_sb[:, 2 * j: 2 * j + 1])
                            else:
                                stg2 = pp.tile([128, 2048], BF16, tag="stg2",
                                               bufs=2)
                                nc.scalar.activation(
                                    stg2[:], ps[:], ACTF.Identity,
                                    bias=pb_sb[:, 2 * j + 1: 2 * j + 2])
                                nc.sync.dma_start(x2d[j][:, cs], stg2[:])

                        lo0 = lo_fetch(0)
                        for c8 in range(8):
                            pin_pass(0, 0, c8, lo0)
                        for c8 in range(8):
                            pin_pass(0, 1, c8, lo0)
                        lo1 = lo_fetch(1)
                        for c8 in range(8):
                            pin_pass(1, 0, c8, lo1)
                        for c8 in range(8):
                            pin_pass(1, 1, c8, lo1)

                # ---- branch + dynconv + gate + pout, pair pool closed ----
                with tc.tile_pool(name="lp", bufs=1) as lp:
                    acc = [lp.tile([128, S], BF16, tag=f"acc{j}",
                                   name=f"acc{j}") for j in range(2)]
                    dg = [lp.tile([128, 128], BF16, tag=f"dg{k}",
                                  name=f"dg{k}") for k in range(9)]
                    psd_ctx = tc.tile_pool(name="ps_d", bufs=1, space="PSUM")
                    psd = psd_ctx.__enter__()

                    def branch(j):
                        x1v = xpv[j][:, 1:129, 1:129]
                        pav = pabv[0]
                        # avgpool2 (sum; 0.25 folded into layer-0 weights)
                        tb = tmpB[:].rearrange("p (h w) -> p h w", h=64)
                        for hf in range(2):
                            xe = x1v[:, 64 * hf: 64 * hf + 64, :].rearrange(
                                "p h (w2 two) -> p h w2 two", two=2)
                            nc.vector.tensor_tensor(
                                tb, xe[:, :, :, 0], xe[:, :, :, 1], op=AL.add)
                            te = tb.rearrange(
                                "p (h2 two) w -> p h2 two w", two=2)
                            nc.vector.tensor_tensor(
                                pav[:, 1 + 32 * hf: 1 + 32 * hf + 32, 1:65],
                                te[:, :, 0, :], te[:, :, 1, :], op=AL.add)
                        cur = 0
                        for l in range(3):
                            nxt = 1 - cur
                            ip = pabv[cur]
                            oi = pabv[nxt][:, 1:65, 1:65]
                            kap = bw_sb[:, l * 9: l * 9 + 9]
                            nc.vector.tensor_scalar(
                                oi, ip[:, 1:65, 1:65], kap[:, 4: 5],
                                bb_sb[:, l: l + 1], op0=AL.mult, op1=AL.add)
                            for t in (0, 1, 2, 3, 5, 6, 7, 8):
                                ky, kx = t // 3, t % 3
                                src = ip[:, ky: ky + 64, kx: kx + 64]
                                nc.vector.tensor_scalar(
                                    tb, src, kap[:, t: t + 1], None,
                                    op0=AL.mult)
                                nc.vector.tensor_tensor(oi, oi, tb, op=AL.add)
                            cur = nxt
                        # maxpool2 -> q32
                        ci = pabv[cur][:, 1:65, 1:65]
                        ce = ci.rearrange("p h (w2 two) -> p h w2 two", two=2)
                        tm = tmpB[:, 0:2048].rearrange("p (h w) -> p h w",
                                                       h=64)
                        nc.vector.tensor_tensor(tm, ce[:, :, :, 0],
                                                ce[:, :, :, 1], op=AL.max)
                        tme = tm.rearrange("p (h2 two) w -> p h2 two w", two=2)
                        q3v = q32[:].rearrange("p (h w) -> p h w", h=32)
                        nc.vector.tensor_tensor(q3v, tme[:, :, 0, :],
                                                tme[:, :, 1, :], op=AL.max)
                        # collapsed 32x32 stack: pooled = <umap, q32>
                        nc.vector.scalar_tensor_tensor(
                            tmpB[:, 2048: 3072], q32[:], 1.0, um_sb[:],
                            op0=AL.mult, op1=AL.mult,
                            accum_out=pooled[:, j: j + 1])
                        nc.vector.tensor_copy(pool16[:, j: j + 1],
                                              pooled[:, j: j + 1])

                    def tok_and_diag(j):
                        psk = psd.tile([128, 512], F32, tag="psk")
                        for k in range(9):
                            nc.tensor.matmul(psk[:, k: k + 1], tokw_sb[k],
                                             pool16[:, j: j + 1],
                                             start=True, stop=True)
                        nc.scalar.copy(kern[:], psk[:, 0:9])
                        nc.vector.tensor_tensor(kern[:], kern[:], tokb_sb[:],
                                                op=AL.add)
                        if dbg:
                            nc.sync.dma_start(k_dbg[j][:, 0:9], kern[:])
                            nc.sync.dma_start(k_dbg[j][:, 9:10],
                                              pooled[:, j: j + 1])
                        for k in range(9):
                            nc.vector.tensor_scalar(
                                dg[k][:], id_sb[:], kern[:, k: k + 1], None,
                                op0=AL.mult)
                        if dbg and j == 0:
                            for k in range(9):
                                nc.sync.dma_start(dg_dbg[k], dg[k][:])
                            nc.sync.dma_start(
                                xi_dbg[j].rearrange("c (h w) -> c h w",
                                                    h=128),
                                xpv[j][:, 1:129, 1:129])

                    def dynconv(j):
                        pe_taps = [t for t in range(9) if t not in TAPS_V]
                        for dc in range(16):
                            r0 = 8 * dc
                            cs = slice(dc * 1024, (dc + 1) * 1024)
                            pd = psd.tile([128, 1024], F32, tag="pd", bufs=2)
                            for ti, t in enumerate(pe_taps):
                                ky, kx = t // 3, t % 3
                                for m in range(2):
                                    rhs = xpv[j][:, ky + r0 + 4 * m:
                                                 ky + r0 + 4 * m + 4,
                                                 kx: kx + 128]
                                    nc.tensor.matmul(
                                        pd[:, m * 512: (m + 1) * 512],
                                        dg[t], rhs,
                                        start=(ti == 0),
                                        stop=(ti == len(pe_taps) - 1))
                            nc.scalar.activation(acc[j][:, cs], pd[:],
                                                 ACTF.Identity,
                                                 bias=dwb_sb[:, :])

                    def vtaps_gate(j):
                        for dc in range(16):
                            r0 = 8 * dc
                            cs = slice(dc * 1024, (dc + 1) * 1024)
                            for t in TAPS_V:
                                ky, kx = t // 3, t % 3
                                vt = lp.tile([128, 1024], BF16, tag="vt",
                                             bufs=2)
                                src = xpv[j][:, ky + r0: ky + r0 + 8,
                                             kx: kx + 128]
                                nc.vector.tensor_scalar(
                                    vt[:].rearrange("p (h w) -> p h w", h=8),
                                    src, kern[:, t: t + 1], None, op0=AL.mult)
                                nc.vector.tensor_tensor(
                                    acc[j][:, cs], acc[j][:, cs], vt[:],
                                    op=AL.add)
                            x2t = lp.tile([128, 1024], BF16, tag="x2t",
                                          bufs=3)
                            nc.sync.dma_start(x2t[:], x2d[j][:, cs])
                            nc.gpsimd.tensor_tensor(
                                acc[j][:, cs], acc[j][:, cs], x2t[:],
                                op=AL.mult)

                    branch(0)
                    tok_and_diag(0)
                    branch(1)  # V work queued behind branch(0); overlaps PE dyn(0)
                    dynconv(0)
                    vtaps_gate(0)
                    tok_and_diag(1)
                    dynconv(1)
                    vtaps_gate(1)
                    if dbg:
                        for j in range(2):
                            nc.sync.dma_start(g_dbg[j], acc[j][:])
                    psd_ctx.__exit__(None, None, None)

                    # pout partials: zab rows 0:64 = W2[1]g0+W2[2]g1 (out t0),
                    # rows 64:128 = W2[0]g0+W2[1]g1 (out t0+1);
                    # zpn rows 0:64 = W2[2]g0 (export t0-1), 64:128 = W2[0]g1.
                    with tc.tile_pool(name="ps_z", bufs=2, space="PSUM") as psz:
                        for zc in range(16):
                            cs = slice(zc * 1024, (zc + 1) * 1024)
                            zt = psz.tile([128, 1024], F32, tag="zt")
                            pt = psz.tile([128, 1024], F32, tag="pt")
                            for c4 in range(2):
                                s5 = slice(c4 * 512, (c4 + 1) * 512)
                                m5 = slice(zc * 1024 + c4 * 512,
                                           zc * 1024 + (c4 + 1) * 512)
                                nc.tensor.matmul(zt[64:128, s5], w2_sb[0],
                                                 acc[0][:, m5],
                                                 start=True, stop=False)
                                nc.tensor.matmul(pt[64:128, s5], w2_sb[0],
                                                 acc[1][:, m5],
                                                 start=True, stop=True)
                            for c4 in range(2):
                                s5 = slice(c4 * 512, (c4 + 1) * 512)
                                m5 = slice(zc * 1024 + c4 * 512,
                                           zc * 1024 + (c4 + 1) * 512)
                                nc.tensor.matmul(zt[64:128, s5], w2_sb[1],
                                                 acc[1][:, m5],
                                                 start=False, stop=True)
                                nc.tensor.matmul(zt[0:64, s5], w2_sb[1],
                                                 acc[0][:, m5],
                                                 start=True, stop=False)
                            for c4 in range(2):
                                s5 = slice(c4 * 512, (c4 + 1) * 512)
                                m5 = slice(zc * 1024 + c4 * 512,
                                           zc * 1024 + (c4 + 1) * 512)
                                nc.tensor.matmul(zt[0:64, s5], w2_sb[2],
                                                 acc[1][:, m5],
                                                 start=False, stop=True)
                                nc.tensor.matmul(pt[0:64, s5], w2_sb[2],
                                                 acc[0][:, m5],
                                                 start=True, stop=True)
                            za = lp.tile([128, 1024], BF16, tag="za", bufs=2)
                            nc.scalar.copy(za[:], zt[:])
                            nc.sync.dma_start(zab[:, cs], za[:])
                            zp = lp.tile([128, 1024], BF16, tag="zp", bufs=2)
                            nc.vector.tensor_copy(zp[:], pt[:])
                            nc.sync.dma_start(zpn[:, cs], zp[:])
    nc.compile()
    return nc


def _xcorr_same(x, k):
    h, w = x.shape
    xp = np.zeros((h + 2, w + 2), x.dtype)
    xp[1: h + 1, 1: w + 1] = x
    y = np.zeros_like(x)
    for a in range(3):
        for b in range(3):
            y += k[a, b] * xp[a: a + h, b: b + w]
    return y


def _prep_weights(ln_w, ln_b, pin_w, pout_w, b1_w, b1_b, b2_w, b2_b, tok_w,
                  tok_b, dw_bias):
    pw = np.asarray(pin_w)[:, :, :, 0, 0].astype(np.float64)  # (256, 64, 3)
    lnw = np.asarray(ln_w).astype(np.float64)
    lnb = np.asarray(ln_b).astype(np.float64)
    W1 = [(pw[:, :, t] * lnw[None, :]).T for t in range(3)]  # (64, 256) each
    s1 = [(pw[:, :, t] * lnw[None, :]).sum(1) for t in range(3)]  # (256,)
    bias1 = [pw[:, :, t] @ lnb for t in range(3)]  # (256,)
    w1p = np.zeros((2, 2, 128, 128), np.float32)
    w1lo = np.zeros((2, 2, 67, 128), np.float32)
    for j in range(2):
        tA, tB = (0, 1) if j == 0 else (1, 2)
        tlo = 2 if j == 0 else 0
        # lo rows 64:67 hold mur slices (j..j+3) = taus (0,1,2) in order
        for oh in range(2):
            ohs = slice(oh * 128, (oh + 1) * 128)
            w1p[j, oh, 0:64] = W1[tA][:, ohs]
            w1p[j, oh, 64:128] = W1[tB][:, ohs]
            w1lo[j, oh, 0:64] = W1[tlo][:, ohs]
            for t in range(3):
                w1lo[j, oh, 64 + t] = -s1[t][ohs]
    bw = np.zeros((128, 27), np.float32)
    bb = np.zeros((128, 3), np.float32)
    b1w = np.asarray(b1_w)[:, :, 0]  # (3, 128, 3, 3)
    for l in range(3):
        bw[:, l * 9: l * 9 + 9] = b1w[l].reshape(128, 9)
        bb[:, l] = np.asarray(b1_b)[l]
    bw[:, 0:9] *= 0.25  # avgpool mean folded into layer-0 taps
    # collapse the 32x32 stack + global mean into <umap, q> + beta
    b2w = np.asarray(b2_w)[:, :, 0].astype(np.float64)  # (3, 128, 3, 3)
    b2b = np.asarray(b2_b).astype(np.float64)  # (3, 128)
    umap = np.zeros((128, 1024), np.float32)
    beta = np.zeros((128,), np.float64)
    for c in range(128):
        g = np.ones((32, 32), np.float64) / 1024.0
        for l in (2, 1, 0):
            g = _xcorr_same(g, np.flip(b2w[l, c]))
        umap[c] = g.reshape(-1)
        z = np.zeros((32, 32), np.float64)
        for l in range(3):
            z = _xcorr_same(z, b2w[l, c]) + b2b[l, c]
        beta[c] = z.mean()
    tokw = np.zeros((9, 128, 128), np.float32)
    tw = np.asarray(tok_w).astype(np.float64)  # (1152, 128)
    for k in range(9):
        tokw[k] = tw[k::9, :].T  # [h, c] = tok_w[c*9+k, h]
    tokb2 = (np.asarray(tok_b).astype(np.float64) + tw @ beta)
    tokb2 = tokb2.reshape(128, 9).astype(np.float32)
    w2 = np.zeros((3, 128, 64), np.float32)
    pow_ = np.asarray(pout_w)[:, :, :, 0, 0]  # (64, 128, 3)
    for t in range(3):
        w2[t] = pow_[:, :, t].T
    dwb = np.asarray(dw_bias).reshape(128, 1).astype(np.float32)
    ident = np.eye(128, dtype=np.float32)
    return (w1p.astype(BF), w1lo.astype(BF), bw, bb, umap.astype(BF),
            tokw.astype(BF), tokb2, w2.astype(BF), dwb, ident.astype(BF),
            np.array([np.asarray(b) for b in bias1]))


def kernel(x, ln_w, ln_b, pin_w, pout_w, b1_w, b1_b, b2_w, b2_b, tok_w, tok_b,
           dw_bias):
    x = np.asarray(x)
    (w1p, w1lo, bw, bb, umap, tokw, tokb2, w2, dwb, ident,
     bias1) = _prep_weights(ln_w, ln_b, pin_w, pout_w, b1_w, b1_b, b2_w, b2_b,
                            tok_w, tok_b, dw_bias)
    if "l1" not in _cache:
        _cache["l1"] = _build()

    xbf = x.astype(BF)  # (B, T, C, H, W)
    in_maps = []
    for i in range(8):
        b, t0 = i // 4, 2 * (i % 4)
        xh = np.zeros((4, C, S), BF)
        for k in range(4):
            t = t0 - 1 + k
            if 0 <= t < T:
                xh[k] = xbf[b, t].reshape(C, S)
        pbias = np.zeros((128, 4), np.float32)
        for j in range(2):
            for oh in range(2):
                s = 0.0
                for tau in range(3):
                    if 0 <= t0 + j - 1 + tau < T:
                        s = s + bias1[tau][oh * 128: (oh + 1) * 128]
                pbias[:, 2 * j + oh] = s
        in_maps.append({
            "xh": xh, "w1p": w1p, "w1lo": w1lo, "pbias": pbias, "bw": bw,
            "bb": bb, "umap": umap, "tokw": tokw, "tokb": tokb2, "dwb": dwb,
            "w2": w2, "ident": ident})
    r1 = run_bass_kernel_spmd(_cache["l1"], in_maps, core_ids=list(range(8)),
                              trace=TRACE)
    PROF["l1"] = r1

    out = x.astype(np.float32).copy()
    for i in range(8):
        b, t0 = i // 4, 2 * (i % 4)
        za = r1.results[i]["zab"].astype(np.float32).reshape(2, C, H, W)
        zp = r1.results[i]["zpn"].astype(np.float32).reshape(2, C, H, W)
        out[b, t0] += za[0]
        out[b, t0 + 1] += za[1]
        if t0 - 1 >= 0:
            out[b, t0 - 1] += zp[0]
        if t0 + 2 < T:
            out[b, t0 + 2] += zp[1]
    return out


# revision 31
# speedup vs baseline: 1.0935x; 1.0516x over previous
"""Trainium2 Bass kernel for nn_CWGDN (dense_cnn): LN -> temporal pin conv ->
dynamic depthwise conv (w/ pooled kernel-generator branch) -> gate -> temporal
pout conv + residual.

Sharding: 16 (b,t) instances over 8 cores (2 each), ONE SPMD launch.
Each core computes gated(t0), gated(t0+1) and the pout partial products its
own gated slices contribute to; the t-halo terms are exported (zpn) and the
host sums partials + residual. No second launch, no halo recompute.

Engine split per core:
  PE : LN stats matmuls, pin matmuls, tok matmuls, the full-res 3x3 dynamic
       depthwise conv as 9 diagonal-weight matmuls w/ PSUM accumulation,
       pout partial matmuls.
  DVE: LN scalar math, x*rsqrt scale, avg/max pools, 5/9 taps of each 64x64
       dwconv layer, gating, diag-weight construction.
  GpSimd: remaining 3+1... 3 taps of each 64x64 layer (independent STT chain).
  Scalar: Square for stats, all PSUM drains (w/ folded biases).

The 32x32 conv stack + global mean collapses on the host into a per-channel
32x32 weight map (linear functional) -> one STT w/ accum on device; its bias
term folds into tok_b.

LayerNorm folds into the pin matmul: x is pre-scaled by r=rsqrt(var+eps)
(per-pixel, via a DMA-broadcast row) and the -mu*r rank-1 terms ride as 3
extra contraction rows; the lnb bias rides in the drain activations.
"""
import sys

sys.path.insert(0, "/opt/trn_rl_repo")

import numpy as np
import ml_dtypes

import concourse.bass as bass
import concourse.tile as tile
from concourse import bacc, mybir
from concourse.bass_utils import run_bass_kernel_spmd

BF = ml_dtypes.bfloat16
F32 = mybir.dt.float32
BF16 = mybir.dt.bfloat16
AL = mybir.AluOpType
ACTF = mybir.ActivationFunctionType

B, T, C, H, W = 2, 8, 64, 128, 128
HID = 128
S = H * W  # 16384
K = 3
EPS = 1e-5
TAPS_V = ()  # dyn-conv taps done on vector engine (rest on PE)

_cache = {}
TRACE = False
PROF = {}


def _build(dbg=False):
    scratch_kind = "ExternalOutput" if dbg else "Internal"
    nc = bacc.Bacc("TRN2", target_bir_lowering=False, debug=False, num_devices=8)
    xh = nc.dram_tensor("xh", [4, C, S], BF16, kind="ExternalInput")
    w1p = nc.dram_tensor("w1p", [2, 2, 128, 128], BF16, kind="ExternalInput")
    w1lo = nc.dram_tensor("w1lo", [2, 2, 67, 128], BF16, kind="ExternalInput")
    pbias = nc.dram_tensor("pbias", [128, 4], F32, kind="ExternalInput")
    bw = nc.dram_tensor("bw", [128, 27], F32, kind="ExternalInput")
    bb = nc.dram_tensor("bb", [128, 3], F32, kind="ExternalInput")
    umap = nc.dram_tensor("umap", [128, 1024], BF16, kind="ExternalInput")
    tokw = nc.dram_tensor("tokw", [9, 128, 128], BF16, kind="ExternalInput")
    tokb = nc.dram_tensor("tokb", [128, 9], F32, kind="ExternalInput")
    dwb = nc.dram_tensor("dwb", [128, 1], F32, kind="ExternalInput")
    w2 = nc.dram_tensor("w2", [3, 128, 64], BF16, kind="ExternalInput")
    ident = nc.dram_tensor("ident", [128, 128], BF16, kind="ExternalInput")
    bdg = nc.dram_tensor("bdg", [9, 128, 128], BF16, kind="ExternalInput")
    zab = nc.dram_tensor("zab", [128, S], BF16, kind="ExternalOutput")
    zpn = nc.dram_tensor("zpn", [128, S], BF16, kind="ExternalOutput")
    # internal DRAM scratch
    scr_sq = nc.dram_tensor("scr_sq", [2, 2, 2 * S], BF16, kind=scratch_kind)
    r_scr = nc.dram_tensor("r_scr", [4, S], BF16, kind=scratch_kind)
    mur_scr = nc.dram_tensor("mur_scr", [4, S], BF16, kind=scratch_kind)
    x2d = nc.dram_tensor("x2d", [2, 128, S], BF16, kind=scratch_kind)
    g_dbg = nc.dram_tensor("g_dbg", [2, 128, S], BF16,
                           kind=scratch_kind) if dbg else None
    k_dbg = nc.dram_tensor("k_dbg", [2, 128, 16], F32,
                           kind=scratch_kind) if dbg else None
    dg_dbg = nc.dram_tensor("dg_dbg", [9, 128, 128], BF16,
                            kind=scratch_kind) if dbg else None
    xi_dbg = nc.dram_tensor("xi_dbg", [2, 128, S], BF16,
                            kind=scratch_kind) if dbg else None

    with tile.TileContext(nc, pool_alloc_mode="queue") as tc:
        with tc.tile_pool(name="wp", bufs=1) as wp:
            w1p_sb, w1lo_sb = [], []
            for j in range(2):
                w1p_sb.append([])
                w1lo_sb.append([])
                for oh in range(2):
                    tp = wp.tile([128, 128], BF16, tag=f"w1p{j}{oh}")
                    nc.sync.dma_start(tp[:], w1p[j, oh])
                    w1p_sb[j].append(tp)
                    tl = wp.tile([67, 128], BF16, tag=f"w1lo{j}{oh}")
                    nc.sync.dma_start(tl[:], w1lo[j, oh])
                    w1lo_sb[j].append(tl)
            pb_sb = wp.tile([128, 4], F32, tag="pb")
            nc.sync.dma_start(pb_sb[:], pbias[:])
            bw_sb = wp.tile([128, 27], F32, tag="bw")
            nc.sync.dma_start(bw_sb[:], bw[:])
            bb_sb = wp.tile([128, 3], F32, tag="bb")
            nc.sync.dma_start(bb_sb[:], bb[:])
            um_sb = wp.tile([128, 1024], BF16, tag="um")
            nc.sync.dma_start(um_sb[:], umap[:])
            tokw_sb = []
            for k in range(9):
                tk = wp.tile([128, 128], BF16, tag=f"tokw{k}")
                nc.sync.dma_start(tk[:], tokw[k])
                tokw_sb.append(tk)
            tokb_sb = wp.tile([128, 9], F32, tag="tokb")
            nc.sync.dma_start(tokb_sb[:], tokb[:])
            dwb_sb = wp.tile([128, 1], F32, tag="dwb")
            nc.sync.dma_start(dwb_sb[:], dwb[:])
            w2_sb = []
            for tau in range(3):
                tw2 = wp.tile([128, 64], BF16, tag=f"w2{tau}")
                nc.sync.dma_start(tw2[:], w2[tau])
                w2_sb.append(tw2)
            id_sb = wp.tile([128, 128], BF16, tag="id")
            nc.sync.dma_start(id_sb[:], ident[:])
            bdg_sb = []
            for k in range(9):
                bk = wp.tile([128, 128], BF16, tag=f"bdg{k}")
                nc.sync.dma_start(bk[:], bdg[k])
                bdg_sb.append(bk)
            i2 = wp.tile([128, 2], BF16, tag="i2")
            nc.gpsimd.memset(i2[:, :], 0.0)
            nc.gpsimd.memset(i2[0:64, 0:1], 1.0)
            nc.gpsimd.memset(i2[64:128, 1:2], 1.0)
            eps_t = wp.tile([128, 1], F32, tag="eps")
            nc.gpsimd.memset(eps_t[:, :], EPS)

            with tc.tile_pool(name="cp0", bufs=1) as cp0:
                x1p = [cp0.tile([128, 130 * 130], BF16, tag=f"x1_{j}",
                                name=f"x1t{j}") for j in range(2)]
                xpv = [x1p[j][:].rearrange("p (h w) -> p h w", h=130)
                       for j in range(2)]
                for j in range(2):
                    nc.gpsimd.memset(xpv[j][:, 0:1, :], 0.0)
                    nc.gpsimd.memset(xpv[j][:, 129:130, :], 0.0)
                    nc.gpsimd.memset(xpv[j][:, 1:129, 0:1], 0.0)
                    nc.gpsimd.memset(xpv[j][:, 1:129, 129:130], 0.0)
                pab = [cp0.tile([128, 66 * 66], BF16, tag=f"pp{n}",
                                name=f"pp{n}") for n in range(2)]
                pabv = [t[:].rearrange("p (h w) -> p h w", h=66) for t in pab]
                for v in pabv:
                    nc.gpsimd.memset(v[:, 0:1, :], 0.0)
                    nc.gpsimd.memset(v[:, 65:66, :], 0.0)
                    nc.gpsimd.memset(v[:, 1:65, 0:1], 0.0)
                    nc.gpsimd.memset(v[:, 1:65, 65:66], 0.0)
                tmpB = cp0.tile([128, 4096], BF16, tag="tmpB")
                q32 = cp0.tile([128, 1024], BF16, tag="q32")
                pooled = cp0.tile([128, 2], F32, tag="pooled")
                pool16 = cp0.tile([128, 2], BF16, tag="pool16")
                kern = cp0.tile([128, 9], F32, tag="kern")

                with tc.tile_pool(name="fp", bufs=1) as fp:
                    pairs = [fp.tile([128, S], BF16, tag=f"pair{p}",
                                     name=f"pair{p}") for p in range(2)]
                    # ---- stats + LN + scale, per pair ----
                    # Per 512-pixel chunk one (2,1024) PSUM tile holds the
                    # channel-sums of x (cols 0:512) and x^2 (cols 512:1024):
                    # same tile_position for both matmuls. scr_sq keeps that
                    # interleaved [S(512)|Q(512)] layout per chunk.
                    with tc.tile_pool(name="sp_", bufs=1) as spool, \
                         tc.tile_pool(name="ps_s", bufs=2, space="PSUM") as psa:
                        for p in range(2):
                            for c8 in range(8):
                                cs = slice(c8 * 2048, (c8 + 1) * 2048)
                                nc.sync.dma_start(
                                    pairs[p][:, cs],
                                    xh[2 * p: 2 * p + 2, :, cs].rearrange(
                                        "s c f -> (s c) f"))
                        for ch in range(16):
                            n0 = ch * 1024
                            for p in range(2):
                                sq = spool.tile([128, 1024], BF16, tag="sq",
                                                bufs=2)
                                nc.vector.tensor_tensor(
                                    sq[:], pairs[p][:, n0: n0 + 1024],
                                    pairs[p][:, n0: n0 + 1024], op=AL.mult)
                                stg = spool.tile([2, 2048], BF16,
                                                 tag="stg", bufs=2)
                                ps = psa.tile([2, 2048], F32, tag="st")
                                for c4 in range(2):
                                    nd = n0 + c4 * 512
                                    nc.tensor.matmul(
                                        ps[:, c4 * 1024: c4 * 1024 + 512],
                                        i2[:], pairs[p][:, nd: nd + 512],
                                        start=True, stop=True)
                                    nc.tensor.matmul(
                                        ps[:, c4 * 1024 + 512:
                                           c4 * 1024 + 1024],
                                        i2[:],
                                        sq[:, c4 * 512: (c4 + 1) * 512],
                                        start=True, stop=True)
                                if (ch + p) % 2 == 0:
                                    nc.scalar.copy(stg[:], ps[:])
                                else:
                                    nc.vector.tensor_copy(stg[:], ps[:])
                                nc.sync.dma_start(
                                    scr_sq[p][:, 2 * n0: 2 * n0 + 2048],
                                    stg[:])
                        for p in range(2):
                            # LN math in pixel-spread layout: sp[q, g, sq, f]
                            # with pixel = (c,q2,f) c=32 chunks, q=(c,q2)
                            sp = spool.tile([128, 512], BF16, tag=f"sp{p}")
                            spv4 = sp[:].rearrange("p (g t f) -> p g t f",
                                                   g=2, t=2)
                            scv = scr_sq[p].rearrange(
                                "g (c t q2 f) -> g c t q2 f",
                                c=32, t=2, q2=4)
                            for g in range(2):
                                for t in range(2):
                                    nc.sync.dma_start(
                                        spv4[:, g, t, :],
                                        scv[g, :, t, :, :])
                            mu = spool.tile([128, 256], F32, tag=f"mu{p}")
                            muv = mu[:].rearrange("p (g f) -> p g f", g=2)
                            nc.vector.tensor_scalar(
                                muv, spv4[:, :, 0, :], 1.0 / 64.0, None,
                                op0=AL.mult)
                            var = spool.tile([128, 256], F32, tag=f"var{p}")
                            varv = var[:].rearrange("p (g f) -> p g f", g=2)
                            nc.vector.tensor_scalar(
                                varv, spv4[:, :, 1, :], 1.0 / 64.0, None,
                                op0=AL.mult)
                            msq = spool.tile([128, 256], F32, tag=f"msq{p}")
                            nc.vector.tensor_tensor(msq[:], mu[:], mu[:],
                                                    op=AL.mult)
                            nc.vector.tensor_tensor(var[:], var[:], msq[:],
                                                    op=AL.subtract)
                            nc.scalar.activation(msq[:], var[:], ACTF.Sqrt,
                                                 bias=eps_t[:, :])  # std
                            nc.vector.reciprocal(var[:], msq[:])  # r
                            r16 = spool.tile([128, 256], BF16, tag=f"r16{p}")
                            nc.vector.tensor_copy(r16[:], var[:])
                            nc.vector.tensor_tensor(mu[:], mu[:], var[:],
                                                    op=AL.mult)  # mu*r
                            m16 = spool.tile([128, 256], BF16, tag=f"m16{p}")
                            nc.vector.tensor_copy(m16[:], mu[:])
                            nc.sync.dma_start(
                                r_scr[2 * p: 2 * p + 2].rearrange(
                                    "g (q f) -> q g f", q=128),
                                r16[:].rearrange("p (g f) -> p g f", g=2))
                            nc.sync.dma_start(
                                mur_scr[2 * p: 2 * p + 2].rearrange(
                                    "g (q f) -> q g f", q=128),
                                m16[:].rearrange("p (g f) -> p g f", g=2))
                        # broadcast r back; scale x in place
                        for c8 in range(8):
                            n0 = c8 * 2048
                            for p in range(2):
                                rb = spool.tile([128, 2048], BF16, tag="rb",
                                                bufs=2)
                                for g in range(2):
                                    nc.sync.dma_start(
                                        rb[g * 64: (g + 1) * 64, :],
                                        r_scr[2 * p + g: 2 * p + g + 1,
                                              n0: n0 + 2048].broadcast_to(
                                                  (64, 2048)))
                                nc.vector.tensor_tensor(
                                    pairs[p][:, n0: n0 + 2048],
                                    pairs[p][:, n0: n0 + 2048],
                                    rb[:], op=AL.mult)

                    # ---- pin matmuls ----
                    # j0-oh0 runs first (full c8 sweep) so x1p[0] completes
                    # early and the V branch phase starts while PE finishes
                    # the remaining pin passes.
                    with tc.tile_pool(name="pp", bufs=1) as pp, \
                         tc.tile_pool(name="ps_b", bufs=2, space="PSUM") as psb:

                        def lo_fetch(j):
                            lo = pp.tile([67, S], BF16, tag="lo")
                            for c8 in range(8):
                                cs = slice(c8 * 2048, (c8 + 1) * 2048)
                                if j == 0:
                                    nc.scalar.dma_start(lo[0:64, cs],
                                                        pairs[1][0:64, cs])
                                else:
                                    nc.scalar.dma_start(lo[0:64, cs],
                                                        pairs[0][64:128, cs])
                                nc.scalar.dma_start(lo[64:67, cs],
                                                    mur_scr[j: j + 3, cs])
                            return lo

                        def pin_pass(j, oh, c8, lo):
                            cs = slice(c8 * 2048, (c8 + 1) * 2048)
                            ps = psb.tile([128, 2048], F32, tag="piny")
                            for c4 in range(4):
                                s5 = slice(c4 * 512, (c4 + 1) * 512)
                                nc.tensor.matmul(
                                    ps[:, s5], w1p_sb[j][oh],
                                    pairs[j][:, c8 * 2048 + c4 * 512:
                                             c8 * 2048 + (c4 + 1) * 512],
                                    start=True, stop=False)
                            for c4 in range(4):
                                s5 = slice(c4 * 512, (c4 + 1) * 512)
                                nc.tensor.matmul(
                                    ps[:, s5], w1lo_sb[j][oh],
                                    lo[:, c8 * 2048 + c4 * 512:
                                       c8 * 2048 + (c4 + 1) * 512],
                                    start=False, stop=True)
                            if oh == 0:
                                r0 = 1 + c8 * 16
                                dst = xpv[j][:, r0: r0 + 16, 1: 1 + 128]
                                nc.scalar.activation(
                                    dst,
                                    ps[:].rearrange("p (h w) -> p h w", h=16),
                                    ACTF.Identity,
                                    bias=pb_sb[:, 2 * j: 2 * j + 1])
                            else:
                                stg2 = pp.tile([128, 2048], BF16, tag="stg2",
                                               bufs=2)
                                nc.scalar.activation(
                                    stg2[:], ps[:], ACTF.Identity,
                                    bias=pb_sb[:, 2 * j + 1: 2 * j + 2])
                                nc.sync.dma_start(x2d[j][:, cs], stg2[:])

                        lo0 = lo_fetch(0)
                        for c8 in range(8):
                            pin_pass(0, 0, c8, lo0)
                        for c8 in range(8):
                            pin_pass(0, 1, c8, lo0)
                        lo1 = lo_fetch(1)
                        for c8 in range(8):
                            pin_pass(1, 0, c8, lo1)
                        for c8 in range(8):
                            pin_pass(1, 1, c8, lo1)

                # ---- branch + dynconv + gate + pout, pair pool closed ----
                with tc.tile_pool(name="lp", bufs=1) as lp:
                    acc = [lp.tile([128, S], BF16, tag=f"acc{j}",
                                   name=f"acc{j}") for j in range(2)]
                    dg = [lp.tile([128, 128], BF16, tag=f"dg{k}",
                                  name=f"dg{k}") for k in range(9)]
                    psd_ctx = tc.tile_pool(name="ps_d", bufs=1, space="PSUM")
                    psd = psd_ctx.__enter__()

                    def branch(j):
                        x1v = xpv[j][:, 1:129, 1:129]
                        pav = pabv[0]
                        # avgpool2 (sum; 0.25 folded into layer-0 weights)
                        tb = tmpB[:].rearrange("p (h w) -> p h w", h=64)
                        for hf in range(2):
                            xe = x1v[:, 64 * hf: 64 * hf + 64, :].rearrange(
                                "p h (w2 two) -> p h w2 two", two=2)
                            nc.vector.tensor_tensor(
                                tb, xe[:, :, :, 0], xe[:, :, :, 1], op=AL.add)
                            te = tb.rearrange(
                                "p (h2 two) w -> p h2 two w", two=2)
                            nc.vector.tensor_tensor(
                                pav[:, 1 + 32 * hf: 1 + 32 * hf + 32, 1:65],
                                te[:, :, 0, :], te[:, :, 1, :], op=AL.add)
                        cur = 0
                        for l in range(2):
                            nxt = 1 - cur
                            ip = pabv[cur]
                            oi = pabv[nxt][:, 1:65, 1:65]
                            kap = bw_sb[:, l * 9: l * 9 + 9]
                            nc.vector.tensor_scalar(
                                oi, ip[:, 1:65, 1:65], kap[:, 4: 5],
                                bb_sb[:, l: l + 1], op0=AL.mult, op1=AL.add)
                            for t in (0, 1, 2, 3, 5, 6, 7, 8):
                                ky, kx = t // 3, t % 3
                                src = ip[:, ky: ky + 64, kx: kx + 64]
                                nc.vector.tensor_scalar(
                                    tb, src, kap[:, t: t + 1], None,
                                    op0=AL.mult)
                                nc.vector.tensor_tensor(oi, oi, tb, op=AL.add)
                            cur = nxt
                        # layer 2 on PE: 9 diag matmuls per 1024-col chunk
                        ip = pabv[cur]
                        nxt = 1 - cur
                        for bc in range(4):
                            r0 = 16 * bc
                            pl = psd.tile([128, 1024], F32, tag="pd", bufs=2)
                            for ti in range(9):
                                ky, kx = ti // 3, ti % 3
                                for m in range(2):
                                    rhs = ip[:, ky + r0 + 8 * m:
                                             ky + r0 + 8 * m + 8,
                                             kx: kx + 64]
                                    nc.tensor.matmul(
                                        pl[:, m * 512: (m + 1) * 512],
                                        bdg_sb[ti], rhs,
                                        start=(ti == 0), stop=(ti == 8))
                            nc.scalar.activation(
                                pabv[nxt][:, 1 + r0: 1 + r0 + 16, 1:65],
                                pl[:].rearrange("p (h w) -> p h w", h=16),
                                ACTF.Identity, bias=bb_sb[:, 2:3])
                        cur = nxt
                        # maxpool2 -> q32
                        ci = pabv[cur][:, 1:65, 1:65]
                        ce = ci.rearrange("p h (w2 two) -> p h w2 two", two=2)
                        tm = tmpB[:, 0:2048].rearrange("p (h w) -> p h w",
                                                       h=64)
                        nc.vector.tensor_tensor(tm, ce[:, :, :, 0],
                                                ce[:, :, :, 1], op=AL.max)
                        tme = tm.rearrange("p (h2 two) w -> p h2 two w", two=2)
                        q3v = q32[:].rearrange("p (h w) -> p h w", h=32)
                        nc.vector.tensor_tensor(q3v, tme[:, :, 0, :],
                                                tme[:, :, 1, :], op=AL.max)
                        # collapsed 32x32 stack: pooled = <umap, q32>
                        nc.vector.scalar_tensor_tensor(
                            tmpB[:, 2048: 3072], q32[:], 1.0, um_sb[:],
                            op0=AL.mult, op1=AL.mult,
                            accum_out=pooled[:, j: j + 1])
                        nc.vector.tensor_copy(pool16[:, j: j + 1],
                                              pooled[:, j: j + 1])

                    def tok_and_diag(j):
                        psk = psd.tile([128, 512], F32, tag="psk")
                        for k in range(9):
                            nc.tensor.matmul(psk[:, k: k + 1], tokw_sb[k],
                                             pool16[:, j: j + 1],
                                             start=True, stop=True)
                        nc.scalar.copy(kern[:], psk[:, 0:9])
                        nc.vector.tensor_tensor(kern[:], kern[:], tokb_sb[:],
                                                op=AL.add)
                        if dbg:
                            nc.sync.dma_start(k_dbg[j][:, 0:9], kern[:])
                            nc.sync.dma_start(k_dbg[j][:, 9:10],
                                              pooled[:, j: j + 1])
                        for k in range(9):
                            nc.vector.tensor_scalar(
                                dg[k][:], id_sb[:], kern[:, k: k + 1], None,
                                op0=AL.mult)
                        if dbg and j == 0:
                            for k in range(9):
                                nc.sync.dma_start(dg_dbg[k], dg[k][:])
                            nc.sync.dma_start(
                                xi_dbg[j].rearrange("c (h w) -> c h w",
                                                    h=128),
                                xpv[j][:, 1:129, 1:129])

                    def dynconv(j):
                        pe_taps = [t for t in range(9) if t not in TAPS_V]
                        for dc in range(16):
                            r0 = 8 * dc
                            cs = slice(dc * 1024, (dc + 1) * 1024)
                            pd = psd.tile([128, 1024], F32, tag="pd", bufs=2)
                            for ti, t in enumerate(pe_taps):
                                ky, kx = t // 3, t % 3
                                for m in range(2):
                                    rhs = xpv[j][:, ky + r0 + 4 * m:
                                                 ky + r0 + 4 * m + 4,
                                                 kx: kx + 128]
                                    nc.tensor.matmul(
                                        pd[:, m * 512: (m + 1) * 512],
                                        dg[t], rhs,
                                        start=(ti == 0),
                                        stop=(ti == len(pe_taps) - 1))
                            nc.scalar.activation(acc[j][:, cs], pd[:],
                                                 ACTF.Identity,
                                                 bias=dwb_sb[:, :])

                    def vtaps_gate(j):
                        for dc in range(16):
                            r0 = 8 * dc
                            cs = slice(dc * 1024, (dc + 1) * 1024)
                            for t in TAPS_V:
                                ky, kx = t // 3, t % 3
                                vt = lp.tile([128, 1024], BF16, tag="vt",
                                             bufs=2)
                                src = xpv[j][:, ky + r0: ky + r0 + 8,
                                             kx: kx + 128]
                                nc.vector.tensor_scalar(
                                    vt[:].rearrange("p (h w) -> p h w", h=8),
                                    src, kern[:, t: t + 1], None, op0=AL.mult)
                                nc.vector.tensor_tensor(
                                    acc[j][:, cs], acc[j][:, cs], vt[:],
                                    op=AL.add)
                            x2t = lp.tile([128, 1024], BF16, tag="x2t",
                                          bufs=3)
                            nc.sync.dma_start(x2t[:], x2d[j][:, cs])
                            nc.gpsimd.tensor_tensor(
                                acc[j][:, cs], acc[j][:, cs], x2t[:],
                                op=AL.mult)

                    branch(0)
                    tok_and_diag(0)
                    dynconv(0)
                    branch(1)  # V ops run concurrent with PE dyn(0)
                    vtaps_gate(0)
                    tok_and_diag(1)
                    dynconv(1)
                    vtaps_gate(1)
                    if dbg:
                        for j in range(2):
                            nc.sync.dma_start(g_dbg[j], acc[j][:])
                    psd_ctx.__exit__(None, None, None)

                    # pout partials: zab rows 0:64 = W2[1]g0+W2[2]g1 (out t0),
                    # rows 64:128 = W2[0]g0+W2[1]g1 (out t0+1);
                    # zpn rows 0:64 = W2[2]g0 (export t0-1), 64:128 = W2[0]g1.
                    with tc.tile_pool(name="ps_z", bufs=2, space="PSUM") as psz:
                        for zc in range(16):
                            cs = slice(zc * 1024, (zc + 1) * 1024)
                            zt = psz.tile([128, 1024], F32, tag="zt")
                            pt = psz.tile([128, 1024], F32, tag="pt")
                            for c4 in range(2):
                                s5 = slice(c4 * 512, (c4 + 1) * 512)
                                m5 = slice(zc * 1024 + c4 * 512,
                                           zc * 1024 + (c4 + 1) * 512)
                                nc.tensor.matmul(zt[64:128, s5], w2_sb[0],
                                                 acc[0][:, m5],
                                                 start=True, stop=False)
                                nc.tensor.matmul(pt[64:128, s5], w2_sb[0],
                                                 acc[1][:, m5],
                                                 start=True, stop=True)
                            for c4 in range(2):
                                s5 = slice(c4 * 512, (c4 + 1) * 512)
                                m5 = slice(zc * 1024 + c4 * 512,
                                           zc * 1024 + (c4 + 1) * 512)
                                nc.tensor.matmul(zt[64:128, s5], w2_sb[1],
                                                 acc[1][:, m5],
                                                 start=False, stop=True)
                                nc.tensor.matmul(zt[0:64, s5], w2_sb[1],
                                                 acc[0][:, m5],
                                                 start=True, stop=False)
                            for c4 in range(2):
                                s5 = slice(c4 * 512, (c4 + 1) * 512)
                                m5 = slice(zc * 1024 + c4 * 512,
                                           zc * 1024 + (c4 + 1) * 512)
                                nc.tensor.matmul(zt[0:64, s5], w2_sb[2],
                                                 acc[1][:, m5],
                                                 start=False, stop=True)
                                nc.tensor.matmul(pt[0:64, s5], w2_sb[2],
                                                 acc[0][:, m5],
                                                 start=True, stop=True)
                            za = lp.tile([128, 1024], BF16, tag="za", bufs=2)
                            nc.scalar.copy(za[:], zt[:])
                            nc.sync.dma_start(zab[:, cs], za[:])
                            zp = lp.tile([128, 1024], BF16, tag="zp", bufs=2)
                            nc.vector.tensor_copy(zp[:], pt[:])
                            nc.sync.dma_start(zpn[:, cs], zp[:])
    nc.compile()
    return nc


def _xcorr_same(x, k):
    h, w = x.shape
    xp = np.zeros((h + 2, w + 2), x.dtype)
    xp[1: h + 1, 1: w + 1] = x
    y = np.zeros_like(x)
    for a in range(3):
        for b in range(3):
            y += k[a, b] * xp[a: a + h, b: b + w]
    return y


def _prep_weights(ln_w, ln_b, pin_w, pout_w, b1_w, b1_b, b2_w, b2_b, tok_w,
                  tok_b, dw_bias):
    pw = np.asarray(pin_w)[:, :, :, 0, 0].astype(np.float64)  # (256, 64, 3)
    lnw = np.asarray(ln_w).astype(np.float64)
    lnb = np.asarray(ln_b).astype(np.float64)
    W1 = [(pw[:, :, t] * lnw[None, :]).T for t in range(3)]  # (64, 256) each
    s1 = [(pw[:, :, t] * lnw[None, :]).sum(1) for t in range(3)]  # (256,)
    bias1 = [pw[:, :, t] @ lnb for t in range(3)]  # (256,)
    w1p = np.zeros((2, 2, 128, 128), np.float32)
    w1lo = np.zeros((2, 2, 67, 128), np.float32)
    for j in range(2):
        tA, tB = (0, 1) if j == 0 else (1, 2)
        tlo = 2 if j == 0 else 0
        # lo rows 64:67 hold mur slices (j..j+3) = taus (0,1,2) in order
        for oh in range(2):
            ohs = slice(oh * 128, (oh + 1) * 128)
            w1p[j, oh, 0:64] = W1[tA][:, ohs]
            w1p[j, oh, 64:128] = W1[tB][:, ohs]
            w1lo[j, oh, 0:64] = W1[tlo][:, ohs]
            for t in range(3):
                w1lo[j, oh, 64 + t] = -s1[t][ohs]
    bw = np.zeros((128, 27), np.float32)
    bb = np.zeros((128, 3), np.float32)
    b1w = np.asarray(b1_w)[:, :, 0]  # (3, 128, 3, 3)
    for l in range(3):
        bw[:, l * 9: l * 9 + 9] = b1w[l].reshape(128, 9)
        bb[:, l] = np.asarray(b1_b)[l]
    bw[:, 0:9] *= 0.25  # avgpool mean folded into layer-0 taps
    # collapse the 32x32 stack + global mean into <umap, q> + beta
    b2w = np.asarray(b2_w)[:, :, 0].astype(np.float64)  # (3, 128, 3, 3)
    b2b = np.asarray(b2_b).astype(np.float64)  # (3, 128)
    umap = np.zeros((128, 1024), np.float32)
    beta = np.zeros((128,), np.float64)
    for c in range(128):
        g = np.ones((32, 32), np.float64) / 1024.0
        for l in (2, 1, 0):
            g = _xcorr_same(g, np.flip(b2w[l, c]))
        umap[c] = g.reshape(-1)
        z = np.zeros((32, 32), np.float64)
        for l in range(3):
            z = _xcorr_same(z, b2w[l, c]) + b2b[l, c]
        beta[c] = z.mean()
    tokw = np.zeros((9, 128, 128), np.float32)
    tw = np.asarray(tok_w).astype(np.float64)  # (1152, 128)
    for k in range(9):
        tokw[k] = tw[k::9, :].T  # [h, c] = tok_w[c*9+k, h]
    tokb2 = (np.asarray(tok_b).astype(np.float64) + tw @ beta)
    tokb2 = tokb2.reshape(128, 9).astype(np.float32)
    w2 = np.zeros((3, 128, 64), np.float32)
    pow_ = np.asarray(pout_w)[:, :, :, 0, 0]  # (64, 128, 3)
    for t in range(3):
        w2[t] = pow_[:, :, t].T
    dwb = np.asarray(dw_bias).reshape(128, 1).astype(np.float32)
    ident = np.eye(128, dtype=np.float32)
    bdg = np.zeros((9, 128, 128), np.float32)
    for t in range(9):
        np.fill_diagonal(bdg[t], bw[:, 18 + t])
    return (w1p.astype(BF), w1lo.astype(BF), bw, bb, umap.astype(BF),
            tokw.astype(BF), tokb2, w2.astype(BF), dwb, ident.astype(BF),
            bdg.astype(BF), np.array([np.asarray(b) for b in bias1]))


def kernel(x, ln_w, ln_b, pin_w, pout_w, b1_w, b1_b, b2_w, b2_b, tok_w, tok_b,
           dw_bias):
    x = np.asarray(x)
    (w1p, w1lo, bw, bb, umap, tokw, tokb2, w2, dwb, ident, bdg,
     bias1) = _prep_weights(ln_w, ln_b, pin_w, pout_w, b1_w, b1_b, b2_w, b2_b,
                            tok_w, tok_b, dw_bias)
    if "l1" not in _cache:
        _cache["l1"] = _build()

    xbf = x.astype(BF)  # (B, T, C, H, W)
    in_maps = []
    for i in range(8):
        b, t0 = i // 4, 2 * (i % 4)
        xh = np.zeros((4, C, S), BF)
        for k in range(4):
            t = t0 - 1 + k
            if 0 <= t < T:
                xh[k] = xbf[b, t].reshape(C, S)
        pbias = np.zeros((128, 4), np.float32)
        for j in range(2):
            for oh in range(2):
                s = 0.0
                for tau in range(3):
                    if 0 <= t0 + j - 1 + tau < T:
                        s = s + bias1[tau][oh * 128: (oh + 1) * 128]
                pbias[:, 2 * j + oh] = s
        in_maps.append({
            "xh": xh, "w1p": w1p, "w1lo": w1lo, "pbias": pbias, "bw": bw,
            "bb": bb, "umap": umap, "tokw": tokw, "tokb": tokb2, "dwb": dwb,
            "w2": w2, "ident": ident, "bdg": bdg})
    r1 = run_bass_kernel_spmd(_cache["l1"], in_maps, core_ids=list(range(8)),
                              trace=TRACE)
    PROF["l1"] = r1

    out = x.astype(np.float32).copy()
    for i in range(8):
        b, t0 = i // 4, 2 * (i % 4)
        za = r1.results[i]["zab"].astype(np.float32).reshape(2, C, H, W)
        zp = r1.results[i]["zpn"].astype(np.float32).reshape(2, C, H, W)
        out[b, t0] += za[0]
        out[b, t0 + 1] += za[1]
        if t0 - 1 >= 0:
            out[b, t0 - 1] += zp[0]
        if t0 + 2 < T:
            out[b, t0 + 2] += zp[1]
    return out
